# revision 40
# baseline (speedup 1.0000x reference)
"""Trainium2 Bass kernel for nn_AlphaEntmax (entmax-bisect over last axis).

Key math fact: the module's ClampMin/ClampMax composition maps any alpha in
[1,2] to exactly 2.0, so the reference computes sparsemax (alpha=2) per row:
    p = relu(x - tau) / sum(relu(x - tau)),  tau s.t. sum(relu(x - tau)) = 1
We solve for tau with Newton/Michelot iterations from tau0 = rowmax - 1
(monotone, finite convergence: tau' = tau + (r-1)/c with r = sum(relu(x-tau)),
c = count(x > tau)), then emit p = relu(x - tau) directly (sum == 1 at
convergence; the reference's own normalize brings both within tolerance).

Iteration schedule (6 slots): "F" slots evaluate r with a real pass; "T"
slots skip the r-pass and use the trapezoid estimate
    r -= step_prev * (c + c_prev)/2
(second-order accurate: exact r would subtract the integral of the count
over the last step). Even tiles run F T F T F F, odd tiles F F T F T F —
the stagger keeps ScalarE fed every iteration. Absmax vs the reference's
50-iteration f32 bisection on the N(0,1) data: 3.5e-3 (gate: 2e-2).

Engine split per tile [128,1024]:
  - load DMA, then one fused VectorE tensor_scalar per tile producing the
    bf16 shadow of -x AND the exact row max via the fp32 accum (reduce-min).
  - r-pass ("P" = partnered): columns [0,SPLIT) on ScalarE
    activation(Relu, bias=-tau, accum_out), columns [SPLIT,K) on a
    custom-authored single-src DVE op (relu(Src0+C1), fused accum add);
    the two partial sums are added in the update. This splits the load
    fractionally and shortens the per-iteration critical path.
  - c-pass: VectorE tensor_scalar(is_lt, reduce-add) on the bf16 -x shadow
    (4x DVE mode). bf16 count errors only perturb the Newton path, never
    the fixed point (step = (r-1)/c is zero iff r==1; r stays exact f32 in
    F slots).
  - per-tile [128,1] update chain on VectorE; emission interleaves tiles in
    waves of 4 and software-pipelines each wave's VectorE-heavy prefix into
    the previous wave's ScalarE-heavy tail so neither engine starves.

Sharding: x [8,16,512,1024] is split along the batch axis, one batch entry
(8192 rows of 1024) per NeuronCore; no cross-core communication.
"""

import numpy as np

B, H, Q, K = 8, 16, 512, 1024
N_CORES = 8
P = 128
ROWS_PER_CORE = (B // N_CORES) * H * Q  # 8192
N_TILES = ROWS_PER_CORE // P  # 64
GROUP = 1  # tiles per lockstep stats group
N_ITER = 6  # iteration slots; "T" slots skip the r-pass (trapezoid estimate)
R_ENG = ["P", "T", "P", "T", "P", "P"]  # even tiles: F T F T F F
R_ENG2 = ["P", "P", "T", "P", "T", "P"]  # odd tiles: F F T F T F (staggered)
C_ENG = ["Vb", "Vb", "Vb", "Vb", "Vb", "Vb"]  # counts: bf16 VectorE 4x
FINAL_ENG = "V"  # "S" | "V" | "SV" (alternate by tile)
WAVE = 4  # groups emitted with interleaved iterations
BUFS = {"xp": 16, "bp": 8, "op": 6, "st": 24}
PIPE_PHASES = True  # emit wave w+1 V-prefix during wave w tail
SCRS_PSUM = False  # ScalarE scratch in PSUM crashes the exec unit on HW
NEGX = True  # bf16 shadow holds -x; counts via is_lt(-x, ntau); no tau tile
PIPE_DEPTH = 1  # waves of phase-A emitted ahead of phase-B
SPLIT = 928  # columns of each "P" r-pass on ScalarE (rest on VectorE custom)

_NC_CACHE = None
_RBR_OP = None


def _register_custom_op():
    """Author a single-src custom DVE op: out=relu(in0+C1), accum=C0+sum(out)."""
    global _RBR_OP
    if _RBR_OP is not None:
        return _RBR_OP
    import concourse.dve_ops as dvo
    from concourse.dve_spec import lower
    from concourse.dve_uop import DveOpSpec

    if "RELU_BIAS_REDUCE" in dvo._SUB_OPCODE_FOR_NAME:
        _RBR_OP = next(o for o in dvo.OPS if o.name == "RELU_BIAS_REDUCE")
        return _RBR_OP

    def _ref(in0, in1, c0, c1, c2):
        b = np.maximum(in0.astype(np.float32) + c1, 0).astype(np.float32)
        return b, c0 + b.reshape(b.shape[0], -1).sum(axis=-1, keepdims=True)

    op = dvo.DveOp(
        "RELU_BIAS_REDUCE",
        dvo.Spec(
            body=dvo.relu(dvo.Src0 + dvo.C1),
            accum=dvo.add,
            accum_init=dvo.C0,
            reference=_ref,
        ),
        subdim=False,
        uops_sha={},
    )
    dvo.OPS.append(op)
    dvo.CUSTOM_DVE_SPECS[op.name] = op.spec
    row = dvo._CUSTOM_DVE_ROW_BASE + len(dvo.OPS) - 1
    assert row < 0x20
    dvo._SUB_OPCODE_FOR_NAME[op.name] = row
    for ver in ("v3", "v4"):
        op.uops_sha[ver] = DveOpSpec(
            name=op.name, opcode=row, uops=lower(op.spec, ver=ver), rd1_en=False
        ).sha(ver)
    _RBR_OP = op
    return op


def _build_nc():
    import concourse.bacc as bacc
    import concourse.mybir as mybir
    from concourse.tile import TileContext

    rbr = _register_custom_op()

    f32 = mybir.dt.float32
    bf16 = mybir.dt.bfloat16
    Alu = mybir.AluOpType
    Act = mybir.ActivationFunctionType

    nc = bacc.Bacc(
        "TRN2", target_bir_lowering=False, debug=False, num_devices=N_CORES
    )
    x_ext = nc.dram_tensor("x", [ROWS_PER_CORE, K], f32, kind="ExternalInput")
    out_ext = nc.dram_tensor("out", [ROWS_PER_CORE, K], f32, kind="ExternalOutput")

    N_GROUPS = N_TILES // GROUP
    GK = GROUP * K
    with TileContext(nc) as tc:
        with (
            tc.tile_pool(name="xp", bufs=BUFS["xp"]) as xp,
            tc.tile_pool(name="bp", bufs=BUFS["bp"]) as bp,
            tc.tile_pool(name="op", bufs=BUFS["op"]) as op,
            tc.tile_pool(name="scr", bufs=1) as scr,
            tc.tile_pool(name="psc", bufs=1, space="PSUM") as psc,
            tc.tile_pool(name="st", bufs=BUFS["st"]) as st,
        ):
            # engine-dedicated scratch (elementwise outputs nobody reads);
            # ScalarE's scratch lives in PSUM (ACT's faster write port)
            scrS = (psc if SCRS_PSUM else scr).tile([P, K], f32, tag="scrS", name="scrS")
            scrV = scr.tile([P, K], f32, tag="scrV")
            scrC = scr.tile([P, K], bf16, tag="scrC")

            # warm the ACT function-table (one-time ~2.7us load) during the
            # first DMA instead of stalling the first real r-pass
            nc.vector.memset(scrV[:, :1], 0.0)
            nc.scalar.activation(scrS[:, :1], scrV[:, :1], Act.Relu)

            def emit_load(g):
                rows = slice(g * GROUP * P, (g + 1) * GROUP * P)
                x_dram = x_ext.ap()[rows, :].rearrange("(t p) k -> p t k", p=P)
                xb = xp.tile([P, GK], f32, tag="xb")
                xbf = bp.tile([P, GK], bf16, tag="xbf")
                st_t = {
                    n: st.tile([P, GROUP], f32, tag=n, name=n)
                    for n in ("mx", "tau", "ntau", "r", "r2", "c0", "c1",
                              "rc", "stp", "csum", "cm", "ntm")
                }
                nc.sync.dma_start(
                    out=xb[:].rearrange("p (t k) -> p t k", t=GROUP), in_=x_dram
                )
                if NEGX:
                    # fused per-tile: bf16 shadow of -x + row max via accum
                    # reduce-min of -x (the accum rides the fp32 datapath
                    # pre-cast, so mn = -max(x) exactly)
                    for i in range(GROUP):
                        nc.vector.tensor_scalar(
                            xbf[:, i * K : (i + 1) * K], xb[:, i * K : (i + 1) * K],
                            -1.0, None, Alu.mult, Alu.min,
                            accum_out=st_t["mx"][:, i : i + 1],
                        )
                    # neg_tau = 1 - max = 1 + mn  (tau tile not needed: counts
                    # compare -x < ntau, except f32 "V" counts which negate)
                    nc.vector.tensor_scalar(
                        st_t["ntau"][:], st_t["mx"][:], 1.0, None, Alu.add
                    )
                else:
                    # fused per-tile: bf16 shadow of x + row max (the accum
                    # reduce-max rides the fp32 datapath pre-cast: exact max)
                    for i in range(GROUP):
                        nc.vector.tensor_scalar(
                            xbf[:, i * K : (i + 1) * K], xb[:, i * K : (i + 1) * K],
                            0.0, None, Alu.add, Alu.max,
                            accum_out=st_t["mx"][:, i : i + 1],
                        )
                    # neg_tau = 1 - mx ; tau = mx - 1
                    nc.vector.tensor_scalar(
                        st_t["ntau"][:], st_t["mx"][:], -1.0, 1.0, Alu.mult, Alu.add
                    )
                    nc.vector.tensor_scalar(
                        st_t["tau"][:], st_t["mx"][:], -1.0, None, Alu.add
                    )
                return xb, xbf, st_t

            def resolve_r_eng(it, t_idx):
                sched = R_ENG if (R_ENG2 is None or t_idx % 2 == 0) else R_ENG2
                r_eng = sched[it]
                if r_eng == "A":
                    r_eng = "S" if t_idx % 2 == 0 else "V"
                elif r_eng == "B":  # quarter on V
                    r_eng = "V" if t_idx % 4 == 3 else "S"
                elif r_eng == "D":  # three-quarter on V
                    r_eng = "S" if t_idx % 4 == 3 else "V"
                return r_eng

            def emit_iter(it, xb, xbf, st_t, g=0):
                tau, ntau = st_t["tau"], st_t["ntau"]
                r = st_t["r"]
                c = st_t["c0"] if it % 2 == 0 else st_t["c1"]
                for i in range(GROUP):
                    xcol = xb[:, i * K : (i + 1) * K]
                    r_i = r[:, i : i + 1]
                    c_i = c[:, i : i + 1]
                    # r = sum(relu(x - tau)); "T" iterations skip the pass and
                    # estimate r in emit_update from the count history instead
                    t_idx = g * GROUP + i
                    r_eng = resolve_r_eng(it, t_idx)
                    if r_eng == "S":
                        nc.scalar.activation(
                            scrS[:], xcol, Act.Relu,
                            bias=ntau[:, i : i + 1], accum_out=r_i,
                        )
                    elif r_eng == "V":
                        nc.vector._custom_dve(
                            rbr, out=scrV[:], in0=xcol, in1=None,
                            s0=0.0, s1=ntau[:, i : i + 1], imm2=0.0,
                            accum_out=r_i,
                        )
                    elif r_eng == "P":
                        # partnered pass: row split across both engines,
                        # partial sums combined in the update
                        nc.scalar.activation(
                            scrS[:, :SPLIT], xcol[:, :SPLIT], Act.Relu,
                            bias=ntau[:, i : i + 1], accum_out=r_i,
                        )
                        nc.vector._custom_dve(
                            rbr, out=scrV[:, : K - SPLIT], in0=xcol[:, SPLIT:],
                            in1=None, s0=0.0, s1=ntau[:, i : i + 1], imm2=0.0,
                            accum_out=st_t["r2"][:, i : i + 1],
                        )
                    if r_eng == "M":
                        # midpoint of the last step: ntau_mid = ntau + stp/2
                        # (ntau was just decreased by stp)
                        nc.vector.scalar_tensor_tensor(
                            st_t["ntm"][:, i : i + 1], st_t["stp"][:, i : i + 1],
                            0.5, ntau[:, i : i + 1], Alu.mult, Alu.add,
                        )
                        nc.vector.tensor_scalar(
                            scrC[:], xbf[:, i * K : (i + 1) * K],
                            st_t["ntm"][:, i : i + 1], None,
                            Alu.is_lt, Alu.add,
                            accum_out=st_t["cm"][:, i : i + 1],
                        )
                    # c = count(x > tau)  (= count(-x < ntau) in NEGX mode)
                    if C_ENG[it] == "Vb":
                        nc.vector.tensor_scalar(
                            scrC[:], xbf[:, i * K : (i + 1) * K],
                            ntau[:, i : i + 1] if NEGX else tau[:, i : i + 1],
                            None,
                            Alu.is_lt if NEGX else Alu.is_gt,
                            Alu.add, accum_out=c_i,
                        )
                    else:
                        nc.vector.tensor_scalar(
                            scrV[:], xcol, tau[:, i : i + 1], None,
                            Alu.is_gt, Alu.add, accum_out=c_i,
                        )


            def emit_update(st_t, it=0, g=0):
                c = st_t["c0"] if it % 2 == 0 else st_t["c1"]
                c_prev = st_t["c1"] if it % 2 == 0 else st_t["c0"]
                # guard c >= 1 (trapezoid steps can overshoot the root by
                # O(step^2), so the pure from-left invariant is not exact)
                nc.vector.tensor_scalar_max(c[:], c[:], 1.0)
                r_eng = resolve_r_eng(it, g * GROUP)
                if r_eng == "P":
                    nc.vector.tensor_tensor(
                        st_t["r"][:], st_t["r"][:], st_t["r2"][:], Alu.add
                    )
                if r_eng == "T":
                    # r estimate: r -= step_prev * (c + c_prev)/2
                    # (exact r would subtract the integral of c over the last
                    #  step; the trapezoid rule is second-order accurate)
                    nc.vector.tensor_tensor(
                        st_t["csum"][:], c[:], c_prev[:], Alu.add
                    )
                    nc.vector.tensor_tensor(
                        st_t["csum"][:], st_t["csum"][:], st_t["stp"][:], Alu.mult
                    )
                    nc.vector.scalar_tensor_tensor(
                        st_t["r"][:], st_t["csum"][:], -0.5, st_t["r"][:],
                        Alu.mult, Alu.add,
                    )
                elif r_eng == "M":
                    # Simpson estimate with the midpoint count:
                    # r -= step_prev * (c_prev + 4*c_mid + c)/6
                    nc.vector.tensor_tensor(
                        st_t["csum"][:], c[:], c_prev[:], Alu.add
                    )
                    nc.vector.scalar_tensor_tensor(
                        st_t["csum"][:], st_t["cm"][:], 4.0, st_t["csum"][:],
                        Alu.mult, Alu.add,
                    )
                    nc.vector.tensor_tensor(
                        st_t["csum"][:], st_t["csum"][:], st_t["stp"][:], Alu.mult
                    )
                    nc.vector.scalar_tensor_tensor(
                        st_t["r"][:], st_t["csum"][:], -1.0 / 6.0, st_t["r"][:],
                        Alu.mult, Alu.add,
                    )
                # step = (r - 1)/c; neg_tau -= step
                nc.vector.reciprocal(st_t["rc"][:], c[:])
                nc.vector.scalar_tensor_tensor(
                    st_t["stp"][:], st_t["r"][:], -1.0, st_t["rc"][:],
                    Alu.add, Alu.mult,
                )
                nc.vector.tensor_tensor(
                    st_t["ntau"][:], st_t["ntau"][:], st_t["stp"][:], Alu.subtract
                )
                # tau (positive) only materialized when a later f32 "V" count
                # needs it
                if (not NEGX) or any(C_ENG[j] == "V" for j in range(it + 1, N_ITER)):
                    nc.vector.tensor_scalar(
                        st_t["tau"][:], st_t["ntau"][:], -1.0, None, Alu.mult
                    )

            def emit_final(g, xb, st_t):
                rows = slice(g * GROUP * P, (g + 1) * GROUP * P)
                o_dram = out_ext.ap()[rows, :].rearrange("(t p) k -> p t k", p=P)
                ob = op.tile([P, GK], f32, tag="ob")
                ntau = st_t["ntau"]
                # p = relu(x + neg_tau); sum(p)==1 at convergence, skip normalize
                for i in range(GROUP):
                    eng = FINAL_ENG if FINAL_ENG != "SV" else ("S" if i % 2 == 0 else "V")
                    if eng == "S":
                        nc.scalar.activation(
                            ob[:, i * K : (i + 1) * K],
                            xb[:, i * K : (i + 1) * K],
                            Act.Relu,
                            bias=ntau[:, i : i + 1],
                        )
                    else:
                        nc.vector.tensor_scalar(
                            ob[:, i * K : (i + 1) * K],
                            xb[:, i * K : (i + 1) * K],
                            ntau[:, i : i + 1], 0.0, Alu.add, Alu.max,
                        )
                nc.sync.dma_start(
                    out=o_dram, in_=ob[:].rearrange("p (t k) -> p t k", t=GROUP)
                )

            # Emit in waves of WAVE groups with iterations interleaved, so an
            # engine always has a sibling group's pass-block to chew on while
            # a group's per-iteration update chain resolves. The VectorE-heavy
            # prefix (load, cast+max, iter 0, update 0) of wave w+1 is emitted
            # during wave w's ScalarE-heavy tail so ScalarE never idles at
            # wave boundaries.
            assert N_GROUPS % WAVE == 0
            n_waves = N_GROUPS // WAVE

            def emit_phase_a(w):
                gs = [w * WAVE + j for j in range(WAVE)]
                state = [emit_load(g) for g in gs]
                for j, (xb, xbf, st_t) in enumerate(state):
                    emit_iter(0, xb, xbf, st_t, g=gs[j])
                for j, (xb, xbf, st_t) in enumerate(state):
                    emit_update(st_t, it=0, g=gs[j])
                return gs, state

            def emit_phase_b(gs, state):
                for it in range(1, N_ITER):
                    for j, (xb, xbf, st_t) in enumerate(state):
                        emit_iter(it, xb, xbf, st_t, g=gs[j])
                    for j, (xb, xbf, st_t) in enumerate(state):
                        emit_update(st_t, it=it, g=gs[j])
                for g, (xb, xbf, st_t) in zip(gs, state):
                    emit_final(g, xb, st_t)

            if PIPE_PHASES:
                depth = PIPE_DEPTH
                from collections import deque
                q = deque(emit_phase_a(w) for w in range(min(depth, n_waves)))
                for w in range(n_waves):
                    cur = q.popleft()
                    if w + depth < n_waves:
                        q.append(emit_phase_a(w + depth))
                    emit_phase_b(*cur)
            else:
                for w in range(n_waves):
                    emit_phase_b(*emit_phase_a(w))

    nc.compile()
    return nc


def _get_nc():
    global _NC_CACHE
    if _NC_CACHE is None:
        _NC_CACHE = _build_nc()
    return _NC_CACHE


def _effective_alpha(alpha):
    # the module's ClampMin/ClampMax pair, verbatim in numpy
    a = np.asarray(alpha, dtype=np.float32)
    a = np.maximum(np.minimum(a, 0.0) - 1.0, 0.0) + 1.0 + np.maximum(a, 0.0)
    a = np.minimum(np.maximum(a, 0.0) - 2.0, 0.0) + 2.0 + np.minimum(a, 0.0)
    return a.astype(np.float32)


def _entmax_bisect_numpy(x, a, n_iter=50):
    """Generic-alpha fallback replicating the reference bisection in f32.
    Never taken for alpha in [1,2] (the clamp maps those to exactly 2.0)."""
    f32 = np.float32
    X = x.reshape(-1, K).astype(np.float32)
    am1 = (np.broadcast_to(a.reshape(1, H), (B, H)).reshape(-1)[
        np.arange(X.shape[0]) // Q
    ].astype(np.float32) - f32(1.0))[:, None]
    Xs = (X * am1).astype(np.float32)

    def p(s):
        pos = s > 0
        return np.where(
            pos, np.power(np.where(pos, s, f32(1.0)), (f32(1.0) / am1)), f32(0.0)
        ).astype(np.float32)

    mx = Xs.max(axis=1, keepdims=True).astype(np.float32)
    tau_lo = (mx - f32(1.0)).astype(np.float32)
    tau_hi = (mx - np.power(f32(1.0 / K), am1)).astype(np.float32)
    f_lo = (p(Xs - tau_lo).sum(axis=1, dtype=np.float32)[:, None] - f32(1.0)).astype(
        np.float32
    )
    dm = (tau_hi - tau_lo).astype(np.float32)
    tau_m = tau_lo.copy()
    for _ in range(n_iter):
        dm = (dm * f32(0.5)).astype(np.float32)
        tau_m = (tau_lo + dm).astype(np.float32)
        f_m = (p(Xs - tau_m).sum(axis=1, dtype=np.float32)[:, None] - f32(1.0)).astype(
            np.float32
        )
        tau_lo = np.where(f_m * f_lo >= 0, tau_m, tau_lo).astype(np.float32)
    pm = p(Xs - tau_m)
    s = pm.sum(axis=1, dtype=np.float32).astype(np.float32)[:, None]
    return (pm / s).astype(np.float32).reshape(B, H, Q, K)


def kernel(**inputs) -> np.ndarray:
    from concourse.bass_utils import run_bass_kernel_spmd

    x = np.ascontiguousarray(np.asarray(inputs["x"], dtype=np.float32))
    alpha = np.asarray(inputs.get("alpha", np.full((1, H), 1.5, np.float32)))
    a_eff = _effective_alpha(alpha)
    if not np.all(a_eff == np.float32(2.0)):
        # out-of-distribution alpha (outside [1,2]): generic slow path
        return _entmax_bisect_numpy(x, a_eff)

    shards = x.reshape(N_CORES, ROWS_PER_CORE, K)
    in_maps = [{"x": shards[i]} for i in range(N_CORES)]

    nc = _get_nc()
    res = run_bass_kernel_spmd(nc, in_maps, core_ids=list(range(N_CORES)))
    out = np.stack([res.results[i]["out"] for i in range(N_CORES)])
    return out.reshape(B, H, Q, K)
